# revision 1
# baseline (speedup 1.0000x reference)
"""Trainium2 Bass kernel for nn_MultiHeadSparseAttention (sparse top-k attention).

Full inputs -> full output; shards (batch, head) pairs across 8 NeuronCores
(2 heads x 2 batches per core; the final out_proj contracts over seq, so each
head's slice of the output is independent -> no collectives needed).

Per-core pipeline, per (b,h) pair:
  xhT[d,s] -> qT/kT[e,s], v[s,e] (PE) -> scores[q,k] per 128-row q-tile (PE,
  causal-masked, triangular-skip) -> exact top-K=819 threshold per row via a
  7-pass count ladder (quantile init + local-density Newton aims + bracket
  bisection) + top-8-below-bracket max8 pick -> gated exp (fp16,
  shift-normalized) -> DMA-xbar transpose -> AV (PE) -> out_proj against
  host-pre-transposed Wo (PE).
"""
import math
import sys

sys.path.insert(0, "/opt/trn_rl_repo")

import numpy as np

import concourse.mybir as mybir
import concourse.tile as tile
from concourse import bacc
import concourse.dve_ops as dve_ops_mod
from concourse.dve_ops import DveOp
from concourse.dve_spec import Spec, Src0, C0, Zero, MaxNeg, select, lower
from concourse.dve_uop import DveOpSpec
from concourse.bass_utils import run_bass_kernel_spmd

F32 = mybir.dt.float32
FP16 = mybir.dt.float16
I32 = mybir.dt.int32
AF = mybir.ActivationFunctionType
ALU = mybir.AluOpType
AXX = mybir.AxisListType.X

B, S, DIM, H, HD = 2, 2048, 2048, 16, 128
K = 819
NT = S // 128          # 16 q-tiles
TSEL = 6               # first tile index containing selection rows
NSEL = NT - TSEL       # 10 selection tiles
MLOW = 8               # rows with n-K < MLOW use bottom extraction
NCORES = 8
HPC = H // NCORES      # heads per core
NPAIR = B * HPC        # 4 (b,h) pairs per core
SCALE = 1.0 / math.sqrt(HD)
NEGBIG = -1e9

# ---------------------------------------------------------------- custom DVE ops


def _register_op(name, body, refn):
    if name in dve_ops_mod.CUSTOM_DVE_SPECS:
        return next(o for o in dve_ops_mod.OPS if o.name == name)
    sp = Spec(body=body, reference=refn)
    op = DveOp.__new__(DveOp)
    object.__setattr__(op, "name", name)
    object.__setattr__(op, "spec", sp)
    object.__setattr__(op, "subdim", False)
    object.__setattr__(op, "perf_en", {})
    shas = {}
    for ver in ("v3", "v4"):
        try:
            s = DveOpSpec(name=name, opcode=0, uops=lower(sp, ver=ver), rd1_en=False)
            shas[ver] = s.sha(ver)
        except Exception:
            pass
    object.__setattr__(op, "uops_sha", shas)
    dve_ops_mod.OPS.append(op)
    dve_ops_mod.CUSTOM_DVE_SPECS[name] = sp
    dve_ops_mod._SUB_OPCODE_FOR_NAME[name] = (
        max(dve_ops_mod._SUB_OPCODE_FOR_NAME.values()) + 1
    )
    return op


_MAXNEG = np.float32(np.finfo(np.float32).min)

OP_BELOW = _register_op(
    "ANT_BELOWKEEP",
    select(Src0 < C0, Src0, MaxNeg),
    lambda in0, in1, s0, s1, imm2: np.where(in0 < s0, in0, _MAXNEG).astype(np.float32),
)
OP_GATE = _register_op(
    "ANT_GATESUB",
    select(Src0 >= C0, Src0 - C0, MaxNeg),
    lambda in0, in1, s0, s1, imm2: np.where(in0 >= s0, in0 - s0, _MAXNEG).astype(
        np.float32
    ),
)
OP_NEGV = _register_op(
    "ANT_NEGVALID",
    select(Src0 >= C0, Zero - Src0, MaxNeg),
    lambda in0, in1, s0, s1, imm2: np.where(in0 >= s0, -in0, _MAXNEG).astype(
        np.float32
    ),
)

# ---------------------------------------------------------------- host tables


def _norm_ppf(p):
    p = np.asarray(p, dtype=np.float64)
    a = [-3.969683028665376e01, 2.209460984245205e02, -2.759285104469687e02,
         1.383577518672690e02, -3.066479806614716e01, 2.506628277459239e00]
    b = [-5.447609879822406e01, 1.615858368580409e02, -1.556989798598866e02,
         6.680131188771972e01, -1.328068155288572e01]
    c = [-7.784894002430293e-03, -3.223964580411365e-01, -2.400758277161838e00,
         -2.549732539343734e00, 4.374664141464968e00, 2.938163982698783e00]
    d = [7.784695709041462e-03, 3.224671290700398e-01, 2.445134137142996e00,
         3.754408661907416e00]
    plow, phigh = 0.02425, 1 - 0.02425
    q = np.where(p < plow, np.sqrt(-2 * np.log(np.clip(p, 1e-300, 1))),
                 np.where(p > phigh, np.sqrt(-2 * np.log(np.clip(1 - p, 1e-300, 1))), 0.0))
    pm = p - 0.5
    r2 = pm * pm
    num = ((((a[0] * r2 + a[1]) * r2 + a[2]) * r2 + a[3]) * r2 + a[4]) * r2 + a[5]
    den = ((((b[0] * r2 + b[1]) * r2 + b[2]) * r2 + b[3]) * r2 + b[4]) * r2 + 1
    mid = num * pm / den
    numl = ((((c[0] * q + c[1]) * q + c[2]) * q + c[3]) * q + c[4]) * q + c[5]
    denl = (((d[0] * q + d[1]) * q + d[2]) * q + d[3]) * q + 1
    tail = numl / denl
    return np.where(p < plow, tail, np.where(p > phigh, -tail, mid))


def _host_tables():
    rows_n = np.arange(S) + 1
    z = _norm_ppf(1 - np.clip(K / rows_n.astype(np.float64), 1e-9, 1 - 1e-9))
    ztab = np.zeros((128, NSEL), np.float32)
    nphi = np.zeros((128, NSEL), np.float32)
    wtab = np.zeros((128, NSEL), np.float32)
    for i, t in enumerate(range(TSEL, NT)):
        r = np.arange(t * 128, (t + 1) * 128)
        ztab[:, i] = z[r]
        nphi[:, i] = rows_n[r] / math.sqrt(2 * math.pi)
        wtab[:, i] = 128 * (t + 1)
    al = np.ones((2, 128, NSEL), np.float32)
    be = np.zeros((2, 128, NSEL), np.float32)
    for r in range(2):
        for i in range(NSEL):
            if (r + i) % 2 == 1:
                al[r, :, i] = 0.5
                be[r, :, i] = wtab[:, i] * 0.5
    jbot = np.zeros((128, 1), np.float32)
    for p in range(128):
        m = TSEL * 128 + p + 1 - K
        if 1 <= m < MLOW:
            jbot[p, 0] = m
    invw16 = np.zeros((128, NT), np.float32)
    invw416 = np.zeros((128, NT), np.float32)
    for t in range(NT):
        W = 128 * (t + 1)
        invw16[:, t] = 1.0 / W
        invw416[:, t] = 1.0 / (W // 4)
    i8cat = np.tile(np.arange(8, dtype=np.float32), NSEL)[None, :].repeat(128, 0)
    mk = np.zeros((128, 2), np.float32)
    for p in range(128):
        n = TSEL * 128 + p + 1
        if n <= K:
            mk[p, 0] = 1.0
        elif n - K < MLOW:
            mk[p, 1] = 1.0
    parts = [("mk", mk), ("ztab", ztab), ("nphi", nphi), ("al0", al[0]), ("be0", be[0]),
             ("al1", al[1]), ("be1", be[1]), ("invw16", invw16),
             ("invw416", invw416), ("jbot", jbot), ("i8cat", i8cat)]
    cols = {}
    off = 0
    for nm, arr in parts:
        cols[nm] = (off, off + arr.shape[1])
        off += arr.shape[1]
    ctab = np.concatenate([a for _, a in parts], axis=1).astype(np.float32)
    return ctab, cols


CTAB, CCOLS = None, None


def _get_ctab():
    global CTAB, CCOLS
    if CTAB is None:
        CTAB, CCOLS = _host_tables()
    return CTAB, CCOLS


# ---------------------------------------------------------------- kernel build

DEBUG_OUTS = False
BODY_REPS = 1

# state-tile column layout [128, SCOLS]
_SL = {}
_off = 0
for _nm, _w in [("acc", NT * 4), ("m1", NT), ("m2", NT), ("sig", NT),
                ("invsig", NT), ("tgate", NT), ("negc", NT), ("zacc", NT),
                ("rz", NT), ("Tc", NSEL), ("Cc", NSEL), ("Thi", NSEL),
                ("Chi", NSEL), ("Tlo", NSEL), ("Clo", NSEL), ("s1", NSEL),
                ("s2", NSEL), ("s3", NSEL), ("negT", NSEL),
                ("w8", NSEL * 8), ("oh", NSEL * 8), ("nm8", 8), ("pk", 10)]:
    _SL[_nm] = (_off, _off + _w)
    _off += _w
SCOLS = _off


def build_nc():
    ctab_np, CC = _get_ctab()
    nc = bacc.Bacc("TRN2", target_bir_lowering=False, debug=False,
                   num_devices=NCORES)

    def din(name, shape, dt=F32):
        return nc.dram_tensor(name, shape, dt, kind="ExternalInput")

    xT = din("xT", [NPAIR, 128, S])
    wqT = din("wqT", [HPC, 128, 128])
    wkT = din("wkT", [HPC, 128, 128])
    wvT = din("wvT", [HPC, 128, 128])
    bqs = din("bqs", [HPC, 128, 1])
    bkc = din("bkc", [HPC, 128, 1])
    bvr = din("bvr", [HPC, 1, 128])
    woT = din("woT", [S, S])
    bor = din("bor", [1, S])
    ctab_d = din("ctab", list(ctab_np.shape))

    y = nc.dram_tensor("y", [B, HPC * 128, S], F32, kind="ExternalOutput")

    dbg = {}
    if DEBUG_OUTS:
        for nm, shp, dt in [
            ("qT", [NPAIR, 128, S], F32), ("kT", [NPAIR, 128, S], F32),
            ("v", [NPAIR, 128, NT, 128], FP16), ("sc15", [NPAIR, 128, S], F32),
            ("stt", [NPAIR, 128, SCOLS], F32),
            ("outh", [NPAIR, 128, NT, 128], F32),
        ]:
            dbg[nm] = nc.dram_tensor("dbg_" + nm, shp, dt, kind="ExternalOutput")

    pairs = [(b, hl) for hl in range(HPC) for b in range(B)]

    with tile.TileContext(nc) as tc:
        with (
            tc.tile_pool(name="const", bufs=1) as cpool,
            tc.tile_pool(name="work", bufs=1) as wpool,
            tc.tile_pool(name="proj", bufs=1) as ppool,
            tc.tile_pool(name="roll", bufs=2) as rpool,
            tc.tile_pool(name="psA", bufs=3, space="PSUM") as psA,
            tc.tile_pool(name="psB", bufs=4, space="PSUM") as psB,
        ):
            ctab = cpool.tile_from(ctab_d[:], name="ctab")

            def ct(nm):
                a, bb = CC[nm]
                return ctab[:, a:bb]

            bo_bc = cpool.tile([128, S], F32, tag="bo_bc")
            bo_row = cpool.tile([1, S], F32, tag="bo_row")
            nc.sync.dma_start(out=bo_row[:], in_=bor[:])
            nc.gpsimd.partition_broadcast(bo_bc[:], bo_row[:])
            wq_sb = cpool.tile([128, HPC * 128], F32, tag="wq_sb")
            wk_sb = cpool.tile([128, HPC * 128], F32, tag="wk_sb")
            wv_sb = cpool.tile([128, HPC * 128], F32, tag="wv_sb")
            bqk_sb = cpool.tile([128, 2 * HPC], F32, tag="bqk_sb")
            bv_bc = cpool.tile([128, HPC * 128], F32, tag="bv_bc")
            for hl in range(HPC):
                hsl = slice(hl * 128, (hl + 1) * 128)
                nc.sync.dma_start(out=wq_sb[:, hsl], in_=wqT[hl])
                nc.sync.dma_start(out=wk_sb[:, hsl], in_=wkT[hl])
                nc.sync.dma_start(out=wv_sb[:, hsl], in_=wvT[hl])
                nc.sync.dma_start(out=bqk_sb[:, hl:hl + 1], in_=bqs[hl])
                nc.sync.dma_start(out=bqk_sb[:, HPC + hl:HPC + hl + 1], in_=bkc[hl])
                bv_row = cpool.tile([1, 128], F32, tag=f"bv_row{hl}", name=f"bv_row{hl}")
                nc.sync.dma_start(out=bv_row[:], in_=bvr[hl])
                nc.gpsimd.partition_broadcast(bv_bc[:, hsl], bv_row[:])

            mki = cpool.tile([128, 2], I32, tag="mki")
            nc.vector.tensor_copy(mki[:], ct("mk"))
            out_h = []
            for pi in range(NPAIR):
                oh_t = wpool.tile([128, NT, 128], F32, tag=f"outh{pi}", name=f"outh{pi}")
                out_h.append(oh_t)
            dump_d = wpool.tile([128, S], F32, tag="dump_dve")
            dump_a = wpool.tile([128, S], F32, tag="dump_act")

            for _rep in range(BODY_REPS):
              for pi, (b, hl) in enumerate(pairs):
                hs = slice(hl * 128, (hl + 1) * 128)
                xhT = ppool.tile([128, S], F32, tag="xhT")
                nc.sync.dma_start(out=xhT[:], in_=xT[pi])

                qT = ppool.tile([128, S], F32, tag="qT")
                kT = ppool.tile([128, S], F32, tag="kT")
                for ch in range(S // 512):
                    cs = slice(ch * 512, (ch + 1) * 512)
                    ps = psA.tile([128, 512], F32, tag="ps512")
                    nc.tensor.matmul(ps[:], wq_sb[:, hs], xhT[:, cs], start=True, stop=True)
                    nc.scalar.activation(qT[:, cs], ps[:], AF.Identity,
                                         bias=bqk_sb[:, hl:hl + 1], scale=SCALE)
                    ps2 = psA.tile([128, 512], F32, tag="ps512")
                    nc.tensor.matmul(ps2[:], wk_sb[:, hs], xhT[:, cs], start=True, stop=True)
                    nc.scalar.activation(kT[:, cs], ps2[:], AF.Identity,
                                         bias=bqk_sb[:, HPC + hl:HPC + hl + 1], scale=1.0)
                v = rpool.tile([128, NT, 128], FP16, tag="v")
                for sb in range(NT):
                    pv = psB.tile([128, 128], F32, tag="ps128")
                    nc.tensor.matmul(pv[:], xhT[:, sb * 128:(sb + 1) * 128],
                                     wv_sb[:, hs], start=True, stop=True)
                    nc.vector.tensor_add(v[:, sb, :], pv[:], bv_bc[:, hs])
                if DEBUG_OUTS:
                    nc.sync.dma_start(out=dbg["qT"][pi], in_=qT[:])
                    nc.sync.dma_start(out=dbg["kT"][pi], in_=kT[:])
                    nc.sync.dma_start(out=dbg["v"][pi], in_=v[:])

                stt = wpool.tile([128, SCOLS], F32, tag="stt")
                ipk = wpool.tile([128, 2 * NSEL], I32, tag="ipk")
                ip1 = ipk[:, 0:NSEL]
                ip2 = ipk[:, NSEL:2 * NSEL]

                def sl(nm):
                    a, bb = _SL[nm]
                    return stt[:, a:bb]

                def slc(nm, i, j=None):
                    a, bb = _SL[nm]
                    if j is None:
                        j = i + 1
                    return stt[:, a + i:a + j]

                nc.vector.memset(sl("acc"), 0.0)
                sc_t = []
                for t in range(NT):
                    sct = wpool.tile([128, 128 * (t + 1)], F32, tag=f"sc{t}", name=f"sc{t}")
                    sc_t.append(sct)

                for t in range(NT):
                    W = 128 * (t + 1)
                    st = sc_t[t]
                    qsl = qT[:, t * 128:(t + 1) * 128]
                    for ch in range((W + 511) // 512):
                        c0, c1 = ch * 512, min((ch + 1) * 512, W)
                        ps = psA.tile([128, 512], F32, tag="ps512")
                        nc.tensor.matmul(ps[:, :c1 - c0], qsl, kT[:, c0:c1],
                                         start=True, stop=True)
                        nc.scalar.activation(st[:, c0:c1], ps[:, :c1 - c0], AF.Copy,
                                             bias=0.0, scale=1.0,
                                             accum_out=slc("acc", t * 4 + ch))
                    nc.scalar.activation(dump_a[:, :W // 4], st[:, 0:W:4], AF.Square,
                                         bias=0.0, scale=1.0,
                                         accum_out=slc("m2", t))
                    nc.gpsimd.affine_select(st[:, t * 128:(t + 1) * 128],
                                            st[:, t * 128:(t + 1) * 128],
                                            pattern=[[-1, 128]],
                                            compare_op=ALU.is_ge,
                                            fill=NEGBIG, base=0, channel_multiplier=1)

                nc.vector.tensor_reduce(sl("m1"),
                                        sl("acc").rearrange("p (t c) -> p t c", c=4),
                                        axis=AXX, op=ALU.add)
                nc.vector.tensor_mul(sl("m1"), sl("m1"), ct("invw16"))
                nc.vector.tensor_mul(sl("m2"), sl("m2"), ct("invw416"))
                nc.vector.tensor_mul(sl("sig"), sl("m1"), sl("m1"))
                nc.vector.tensor_sub(sl("sig"), sl("m2"), sl("sig"))
                nc.vector.tensor_scalar_max(sl("sig"), sl("sig"), 1e-6)
                nc.scalar.activation(sl("sig"), sl("sig"), AF.Sqrt, bias=0.0, scale=1.0)
                nc.vector.reciprocal(sl("invsig"), sl("sig"))

                nc.vector.tensor_scalar(sl("tgate"), sl("sig"), -4.0, None, op0=ALU.mult)
                nc.vector.tensor_add(sl("tgate"), sl("tgate"), sl("m1"))
                nc.vector.tensor_scalar(sl("negc"), sl("sig"), 9.0, -10.5,
                                        op0=ALU.mult, op1=ALU.add)
                nc.vector.tensor_scalar_max(sl("negc"), sl("negc"), 6.0)
                nc.vector.tensor_scalar_mul(sl("negc"), sl("negc"), -1.0)

                m1s = slc("m1", TSEL, NT)
                sigs = slc("sig", TSEL, NT)
                invsigs = slc("invsig", TSEL, NT)
                Tc, Cc = sl("Tc"), sl("Cc")
                Thi, Chi, Tlo, Clo = sl("Thi"), sl("Chi"), sl("Tlo"), sl("Clo")
                s1, s2, s3, negT = sl("s1"), sl("s2"), sl("s3"), sl("negT")

                nc.vector.tensor_scalar(Thi, sigs, 3.0, None, op0=ALU.mult)
                nc.vector.tensor_add(Thi, Thi, m1s)
                nc.vector.memset(Chi, 0.0)
                nc.vector.tensor_scalar(Tlo, sigs, -4.0, None, op0=ALU.mult)
                nc.vector.tensor_add(Tlo, Tlo, m1s)
                nc.vector.memset(Clo, float(S))
                nc.vector.tensor_mul(Tc, sigs, ct("ztab"))
                nc.vector.tensor_add(Tc, Tc, m1s)

                def emit_count(rung):
                    nc.vector.tensor_scalar_mul(negT, Tc, -1.0)
                    for i, t in enumerate(range(TSEL, NT)):
                        W = 128 * (t + 1)
                        st = sc_t[t]
                        if (rung + i) % 2 == 0:
                            nc.vector.tensor_scalar(
                                dump_d[:, :W], st[:], slc("Tc", i), 0.0,
                                op0=ALU.is_ge, op1=ALU.add, accum_out=slc("Cc", i))
                        else:
                            nc.scalar.activation(
                                dump_a[:, :W], st[:], AF.Sign,
                                bias=slc("negT", i), scale=1.0, accum_out=slc("Cc", i))
                    p = rung % 2
                    nc.vector.tensor_mul(Cc, Cc, ct("al" + str(p)))
                    nc.vector.tensor_add(Cc, Cc, ct("be" + str(p)))
                    # ceil-fix for ACT sign-count ties: C += 2*(C - rnd(C))^2
                    nc.vector.tensor_scalar(s3, Cc, 8388608.0, 8388608.0,
                                            op0=ALU.add, op1=ALU.subtract)
                    nc.vector.tensor_sub(s3, Cc, s3)
                    nc.vector.tensor_mul(s3, s3, s3)
                    nc.vector.tensor_scalar_mul(s3, s3, 2.0)
                    nc.vector.tensor_add(Cc, Cc, s3)

                def emit_update():
                    nc.vector.tensor_scalar(ip1, Cc, float(K), None, op0=ALU.is_le)
                    nc.vector.tensor_tensor(ip2, Cc, Chi, op=ALU.is_ge)
                    nc.vector.tensor_tensor(ip1, ip1, ip2, op=ALU.logical_and)
                    nc.vector.copy_predicated(Thi, ip1, Tc)
                    nc.vector.copy_predicated(Chi, ip1, Cc)
                    nc.vector.tensor_scalar(ip1, Cc, float(K), None, op0=ALU.is_gt)
                    nc.vector.tensor_tensor(ip2, Cc, Clo, op=ALU.is_le)
                    nc.vector.tensor_tensor(ip1, ip1, ip2, op=ALU.logical_and)
                    nc.vector.copy_predicated(Tlo, ip1, Tc)
                    nc.vector.copy_predicated(Clo, ip1, Cc)

                def emit_newton(aim):
                    nc.vector.tensor_sub(s1, Tc, m1s)
                    nc.vector.tensor_mul(s1, s1, invsigs)
                    nc.vector.tensor_mul(s1, s1, s1)
                    nc.scalar.activation(s1, s1, AF.Exp, bias=0.0, scale=-0.5)
                    nc.vector.tensor_mul(s1, s1, ct("nphi"))
                    nc.vector.tensor_mul(s1, s1, invsigs)
                    nc.vector.tensor_scalar_max(s1, s1, 15.0)
                    nc.vector.reciprocal(s1, s1)
                    nc.vector.tensor_scalar(s2, Cc, float(aim), None, op0=ALU.subtract)
                    nc.vector.tensor_mul(s2, s2, s1)
                    nc.vector.tensor_add(s3, Tc, s2)

                def emit_clamp_into_Tc():
                    nc.vector.tensor_tensor(s3, s3, Tlo, op=ALU.max)
                    nc.vector.tensor_tensor(s3, s3, Thi, op=ALU.min)
                    nc.vector.tensor_tensor(ip1, s3, Tlo, op=ALU.is_le)
                    nc.vector.tensor_tensor(ip2, s3, Thi, op=ALU.is_ge)
                    nc.vector.tensor_tensor(ip1, ip1, ip2, op=ALU.logical_or)
                    nc.vector.tensor_tensor(s2, Thi, Tlo, op=ALU.add)
                    nc.vector.tensor_scalar_mul(s2, s2, 0.5)
                    nc.vector.copy_predicated(s3, ip1, s2)
                    nc.vector.tensor_copy(Tc, s3)

                emit_count(0)
                emit_update()
                for r, aim in enumerate([K, K - 5, K - 3], start=1):
                    emit_newton(aim)
                    emit_clamp_into_Tc()
                    emit_count(r)
                    emit_update()
                for r in range(4, 7):
                    nc.vector.tensor_tensor(s3, Thi, Tlo, op=ALU.add)
                    nc.vector.tensor_scalar_mul(s3, s3, 0.5)
                    nc.vector.tensor_copy(Tc, s3)
                    emit_count(r)
                    emit_update()

                w8 = sl("w8")
                for i, t in enumerate(range(TSEL, NT)):
                    W = 128 * (t + 1)
                    nc.vector._custom_dve(OP_BELOW, out=dump_d[:, :W], in0=sc_t[t][:],
                                          s0=slc("Thi", i))
                    nc.vector.max(out=slc("w8", i * 8, i * 8 + 8), in_=dump_d[:, :W])
                Wt6 = 128 * (TSEL + 1)
                nc.vector._custom_dve(OP_NEGV, out=dump_a[:, :Wt6], in0=sc_t[TSEL][:],
                                      s0=-1e8)
                nc.vector.max(out=sl("nm8"), in_=dump_a[:, :Wt6])

                nc.vector.tensor_scalar(s1, Chi, -1.0, float(K - 1),
                                        op0=ALU.mult, op1=ALU.add)
                nc.vector.tensor_scalar_max(s1, s1, 0.0)
                nc.vector.tensor_scalar_min(s1, s1, 7.0)
                oh = sl("oh")
                nc.vector.tensor_tensor(
                    oh.rearrange("p (t e) -> p t e", e=8),
                    ct("i8cat").rearrange("p (t e) -> p t e", e=8),
                    s1.to_broadcast([128, NSEL, 8]), op=ALU.is_equal)
                nc.vector.tensor_mul(oh, oh, w8)
                nc.vector.tensor_reduce(s2, oh.rearrange("p (t e) -> p t e", e=8),
                                        axis=AXX, op=ALU.add)
                nc.vector.tensor_scalar(ip1, Chi, float(K) - 0.5, None, op0=ALU.is_ge)
                nc.vector.copy_predicated(s2, ip1, Thi)
                nc.vector.tensor_copy(slc("tgate", TSEL, NT), s2)
                nc.vector.memset(slc("negc", TSEL, NT), -6.0)

                # keep-all rows: tgate = m1 - 4 sig ; negc = -max(6, 9 sig - 10.5)
                nc.vector.tensor_scalar(slc("pk", 0), slc("sig", TSEL), -4.0, None,
                                        op0=ALU.mult)
                nc.vector.tensor_add(slc("pk", 0), slc("pk", 0), slc("m1", TSEL))
                nc.vector.copy_predicated(slc("tgate", TSEL), mki[:, 0:1],
                                          slc("pk", 0))
                nc.vector.tensor_scalar(slc("pk", 1), slc("sig", TSEL), 9.0, -10.5,
                                        op0=ALU.mult, op1=ALU.add)
                nc.vector.tensor_scalar_max(slc("pk", 1), slc("pk", 1), 6.0)
                nc.vector.tensor_scalar_mul(slc("pk", 1), slc("pk", 1), -1.0)
                nc.vector.copy_predicated(slc("negc", TSEL), mki[:, 0:1],
                                          slc("pk", 1))
                # bottom rows: tgate = -nm8[jbot]
                nc.vector.tensor_scalar(slc("pk", 2, 10), ct("i8cat")[:, 0:8],
                                        ct("jbot"), None, op0=ALU.is_equal)
                nc.vector.tensor_mul(slc("pk", 2, 10), slc("pk", 2, 10), sl("nm8"))
                nc.vector.tensor_reduce(slc("pk", 0), slc("pk", 2, 10),
                                        axis=AXX, op=ALU.add)
                nc.vector.tensor_scalar_mul(slc("pk", 0), slc("pk", 0), -1.0)
                nc.vector.copy_predicated(slc("tgate", TSEL), mki[:, 1:2],
                                          slc("pk", 0))

                if DEBUG_OUTS:
                    nc.sync.dma_start(out=dbg["stt"][pi], in_=stt[:])
                    nc.sync.dma_start(out=dbg["sc15"][pi], in_=sc_t[15][:])

                for t in range(NT):
                    W = 128 * (t + 1)
                    st = sc_t[t]
                    nc.vector._custom_dve(OP_GATE, out=st[:], in0=st[:],
                                          s0=slc("tgate", t))
                    et = rpool.tile([128, S], FP16, tag="et")
                    nc.scalar.activation(et[:, :W], st[:], AF.Exp,
                                         bias=slc("negc", t), scale=1.0,
                                         accum_out=slc("zacc", t))
                    aT = rpool.tile([128, NT, 128], FP16, tag="aT")
                    nc.sync.dma_start_transpose(aT[:, :t + 1, :], et[:, :W])
                    nc.vector.reciprocal(slc("rz", t), slc("zacc", t))
                    po = psB.tile([128, 128], F32, tag="ps128")
                    for kb in range(t + 1):
                        nc.tensor.matmul(po[:], aT[:, kb, :], v[:, kb, :],
                                         start=(kb == 0), stop=(kb == t))
                    nc.scalar.activation(out_h[pi][:, t, :], po[:], AF.Identity,
                                         bias=0.0, scale=slc("rz", t))
                if DEBUG_OUTS:
                    nc.sync.dma_start(out=dbg["outh"][pi], in_=out_h[pi][:])

              NCH = 8
              CW = S // NCH
              for ch in range(NCH):
                  cs = slice(ch * CW, (ch + 1) * CW)
                  wo_t = ppool.tile([128, NT, CW], F32, tag="wo_t")
                  nc.sync.dma_start(out=wo_t[:],
                                    in_=woT[:, cs].rearrange("(bb p) c -> p bb c", p=128))
                  for pi, (b, hl) in enumerate(pairs):
                      pg = psA.tile([128, 512], F32, tag="ps512")
                      for sb in range(NT):
                          nc.tensor.matmul(pg[:, :CW], out_h[pi][:, sb, :],
                                           wo_t[:, sb, :],
                                           start=(sb == 0), stop=(sb == NT - 1))
                      yt = rpool.tile([128, CW], F32, tag="yt")
                      nc.vector.tensor_add(yt[:], pg[:, :CW], bo_bc[:, cs])
                      nc.sync.dma_start(out=y[b, hl * 128:(hl + 1) * 128, cs], in_=yt[:])

    nc.compile()
    return nc, dbg


# ---------------------------------------------------------------- host side

_NC_CACHE = {}


def get_nc():
    if "nc" not in _NC_CACHE:
        _NC_CACHE["nc"] = build_nc()
    return _NC_CACHE["nc"]


def host_prep(x, Wq, Wk, Wv, bq, bk, bv, Wo, bo):
    ctab, _ = _get_ctab()
    woT = np.ascontiguousarray(Wo.T.astype(np.float32))
    in_maps = []
    pairs = [(b, hl) for hl in range(HPC) for b in range(B)]
    for c in range(NCORES):
        heads = [HPC * c + i for i in range(HPC)]
        xTs = np.empty((NPAIR, 128, S), np.float32)
        for pi, (b, hl) in enumerate(pairs):
            h = heads[hl]
            xTs[pi] = np.ascontiguousarray(
                x[b, :, h * HD:(h + 1) * HD].T.astype(np.float32))
        m = dict(
            xT=xTs,
            wqT=np.ascontiguousarray(
                np.stack([Wq[h].T for h in heads]).astype(np.float32)),
            wkT=np.ascontiguousarray(
                np.stack([Wk[h].T for h in heads]).astype(np.float32)),
            wvT=np.ascontiguousarray(
                np.stack([Wv[h].T for h in heads]).astype(np.float32)),
            bqs=np.ascontiguousarray(
                (np.stack([bq[h] for h in heads]) * SCALE)[:, :, None].astype(np.float32)),
            bkc=np.ascontiguousarray(
                np.stack([bk[h] for h in heads])[:, :, None].astype(np.float32)),
            bvr=np.ascontiguousarray(
                np.stack([bv[h] for h in heads])[:, None, :].astype(np.float32)),
            woT=woT,
            bor=np.ascontiguousarray(bo[None, :].astype(np.float32)),
            ctab=ctab,
        )
        in_maps.append(m)
    return in_maps


def kernel(x, causal_mask, Wq, Wk, Wv, bq, bk, bv, Wo, bo):
    nc, _dbg = get_nc()
    in_maps = host_prep(np.asarray(x), np.asarray(Wq), np.asarray(Wk),
                        np.asarray(Wv), np.asarray(bq), np.asarray(bk),
                        np.asarray(bv), np.asarray(Wo), np.asarray(bo))
    res = run_bass_kernel_spmd(nc, in_maps, list(range(NCORES)))
    y = np.empty((B, DIM, S), np.float32)
    for c in range(NCORES):
        y[:, c * HPC * HD:(c + 1) * HPC * HD, :] = res.results[c]["y"]
    return y



# revision 50
# speedup vs baseline: 2.6183x; 2.6183x over previous
"""Trainium2 Bass kernel for nn_MultiHeadSparseAttention (sparse top-k attention).

Full inputs -> full output; shards (batch, head) pairs across 8 NeuronCores
(2 heads x 2 batches per core; the final out_proj contracts over seq, so each
head's slice of the output is independent -> no collectives needed).

Engine plan (per core, 4 (b,h) pairs):
  PE   : all matmuls in bf16 (QKV proj, scores, AV, out_proj) + identity-
         matmul trick to add the causal -1e9 upper-tri into the diag block
  ACT  : PSUM->SBUF score copies (bf16 out), m2 (pre-diag window), exp, sqrt
  DVE  : count-ladder passes in bf16 4x mode, mask is_ge, PSUM-side small ops
         (m1 extract, v bias-add, rescale, y bias-add), reciprocals
  Pool : ladder bracket/Newton small ops (arithmetic predication), mask mult
  SP   : DMAs + fp16 attn transpose

Top-k threshold: 5 counted rungs (z-init + 3 Newton + 1 bisect) -> Thi
(count<=K, deficit <~10; validated ~+1e-4 rel-err on top of bf16's 5e-3).
"""
import math
import sys

sys.path.insert(0, "/opt/trn_rl_repo")

import numpy as np
import ml_dtypes

import concourse.mybir as mybir
import concourse.tile as tile
from concourse import bacc
from concourse.bass_utils import run_bass_kernel_spmd

F32 = mybir.dt.float32
BF16 = mybir.dt.bfloat16
FP16 = mybir.dt.float16
AF = mybir.ActivationFunctionType
ALU = mybir.AluOpType
AXX = mybir.AxisListType.X

B, S, DIM, H, HD = 2, 2048, 2048, 16, 128
K = 819
NT = S // 128          # 16 q-tiles
TSEL = 6               # first tile index containing selection rows
NSEL = NT - TSEL       # 10 selection tiles
NCORES = 8
HPC = H // NCORES      # heads per core
NPAIR = B * HPC        # 4 (b,h) pairs per core
SCALE = 1.0 / math.sqrt(HD)
NEGBIG = -1e9
NEWTON_AIMS = [K, K - 5, K - 3]   # counted rungs 1..3
NBISECT = 1                       # counted rungs after Newton
CW = 128                          # out_proj chunk width
NCH = S // CW
M2_STRIDE = 8
MULT_DVE_TILES = set()            # mask-mult tiles forced onto DVE (rest Pool)
COPY_DVE_TILES = set(range(6))    # score-copy tiles routed to DVE (rest ACT)

# ---------------------------------------------------------------- host tables


def _norm_ppf(p):
    p = np.asarray(p, dtype=np.float64)
    a = [-3.969683028665376e01, 2.209460984245205e02, -2.759285104469687e02,
         1.383577518672690e02, -3.066479806614716e01, 2.506628277459239e00]
    b = [-5.447609879822406e01, 1.615858368580409e02, -1.556989798598866e02,
         6.680131188771972e01, -1.328068155288572e01]
    c = [-7.784894002430293e-03, -3.223964580411365e-01, -2.400758277161838e00,
         -2.549732539343734e00, 4.374664141464968e00, 2.938163982698783e00]
    d = [7.784695709041462e-03, 3.224671290700398e-01, 2.445134137142996e00,
         3.754408661907416e00]
    plow, phigh = 0.02425, 1 - 0.02425
    q = np.where(p < plow, np.sqrt(-2 * np.log(np.clip(p, 1e-300, 1))),
                 np.where(p > phigh, np.sqrt(-2 * np.log(np.clip(1 - p, 1e-300, 1))), 0.0))
    pm = p - 0.5
    r2 = pm * pm
    num = ((((a[0] * r2 + a[1]) * r2 + a[2]) * r2 + a[3]) * r2 + a[4]) * r2 + a[5]
    den = ((((b[0] * r2 + b[1]) * r2 + b[2]) * r2 + b[3]) * r2 + b[4]) * r2 + 1
    mid = num * pm / den
    numl = ((((c[0] * q + c[1]) * q + c[2]) * q + c[3]) * q + c[4]) * q + c[5]
    denl = (((d[0] * q + d[1]) * q + d[2]) * q + d[3]) * q + 1
    tail = numl / denl
    return np.where(p < plow, tail, np.where(p > phigh, -tail, mid))


def _host_tables():
    rows_n = np.arange(S) + 1
    z = _norm_ppf(1 - np.clip(K / rows_n.astype(np.float64), 1e-9, 1 - 1e-9))
    ztab = np.zeros((128, NSEL), np.float32)
    densz = np.zeros((128, NSEL), np.float32)
    for i, t in enumerate(range(TSEL, NT)):
        r = np.arange(t * 128, (t + 1) * 128)
        ztab[:, i] = z[r]
        # n * phi(z): Gaussian density (x sigma) at the init quantile;
        # Newton rungs reuse it instead of recomputing exp on ACT each rung
        densz[:, i] = rows_n[r] * np.exp(-0.5 * z[r] ** 2) / math.sqrt(2 * math.pi)
    # m2 normalization: 1/nsamp for the pre-diag stride-M2_STRIDE window
    m2scl = np.zeros((128, NT), np.float32)
    for t in range(1, NT):
        m2scl[:, t] = 1.0 / ((t * 128) // M2_STRIDE)
    parts = [("ztab", ztab), ("densz", densz), ("m2scl", m2scl)]
    cols = {}
    off = 0
    for nm, arr in parts:
        cols[nm] = (off, off + arr.shape[1])
        off += arr.shape[1]
    ctab = np.concatenate([a for _, a in parts], axis=1).astype(np.float32)
    return ctab, cols


CTAB, CCOLS = None, None


def _get_ctab():
    global CTAB, CCOLS
    if CTAB is None:
        CTAB, CCOLS = _host_tables()
    return CTAB, CCOLS


# ---------------------------------------------------------------- kernel build

BODY_REPS = 1

# state-tile column layout [128, SCOLS] f32
_SL = {}
_off = 0
for _nm, _w in [("m1", NT), ("m2", NT), ("sig", NT), ("invsig", NSEL),
                ("tgate", NT), ("negc", NT), ("bias", NT), ("rz", NT),
                ("Tc", NSEL), ("Cc", NSEL), ("Thi", NSEL), ("Chi", NSEL),
                ("Tlo", NSEL), ("Clo", NSEL), ("s1", NSEL), ("s2", NSEL),
                ("s3", NSEL), ("p1", NSEL), ("p2", NSEL), ("dinv", NSEL),
                ("zc", 1)]:
    _SL[_nm] = (_off, _off + _w)
    _off += _w
SCOLS = _off


def build_nc():
    ctab_np, CC = _get_ctab()
    nc = bacc.Bacc("TRN2", target_bir_lowering=False, debug=False,
                   num_devices=NCORES)

    def din(name, shape, dt=F32):
        return nc.dram_tensor(name, shape, dt, kind="ExternalInput")

    xT = din("xT", [NPAIR, 128, S], BF16)
    wqT = din("wqT", [HPC, 128, 128], BF16)
    wkT = din("wkT", [HPC, 128, 128], BF16)
    wvT = din("wvT", [HPC, 128, 128], BF16)
    bqs = din("bqs", [HPC, 128, 1])
    bkc = din("bkc", [HPC, 128, 1])
    bvr = din("bvr", [HPC, 1, 128])
    # host-prerearranged: woTr[p, ch, bb, c] = Wo.T[bb*128+p, ch*CW+c]
    # -> a chunk load [128, NT, CW] is one contiguous descriptor per partition
    woT = din("woT", [128, NCH, NT, CW], BF16)
    bor = din("bor", [1, S], BF16)
    identd = din("ident", [128, 128], BF16)
    negud = din("negu", [128, 128], BF16)
    ctab_d = din("ctab", list(ctab_np.shape))

    y = nc.dram_tensor("y", [B, HPC * 128, S], F32, kind="ExternalOutput")

    pairs = [(b, hl) for hl in range(HPC) for b in range(B)]

    with tile.TileContext(nc) as tc:
        with (
            tc.tile_pool(name="const", bufs=1) as cpool,
            tc.tile_pool(name="state", bufs=1) as spool,
            tc.tile_pool(name="sc", bufs=3) as scpool,
            tc.tile_pool(name="proj", bufs=2) as ppool,
            tc.tile_pool(name="roll", bufs=2) as rpool,
            tc.tile_pool(name="vpool", bufs=3) as vpool,
            tc.tile_pool(name="oh", bufs=3) as ohpool,
            tc.tile_pool(name="msk", bufs=1) as mskpool,
            tc.tile_pool(name="wop", bufs=2) as wopool,
            tc.tile_pool(name="psA", bufs=3, space="PSUM") as psA,
            tc.tile_pool(name="psB", bufs=4, space="PSUM") as psB,
        ):
            ctab = cpool.tile_from(ctab_d[:], name="ctab")

            def ct(nm):
                a, bb = CC[nm]
                return ctab[:, a:bb]

            ident = cpool.tile([128, 128], BF16, tag="ident")
            negu = cpool.tile([128, 128], BF16, tag="negu")
            nc.sync.dma_start(out=ident[:], in_=identd[:])
            nc.sync.dma_start(out=negu[:], in_=negud[:])
            bo_bc = cpool.tile([128, S], BF16, tag="bo_bc")
            bo_row = cpool.tile([1, S], BF16, tag="bo_row")
            nc.sync.dma_start(out=bo_row[:], in_=bor[:])
            nc.gpsimd.partition_broadcast(bo_bc[:], bo_row[:])
            wq_sb = cpool.tile([128, HPC * 128], BF16, tag="wq_sb")
            wk_sb = cpool.tile([128, HPC * 128], BF16, tag="wk_sb")
            wv_sb = cpool.tile([128, HPC * 128], BF16, tag="wv_sb")
            bqk_sb = cpool.tile([128, 2 * HPC], F32, tag="bqk_sb")
            bv_bc = cpool.tile([128, HPC * 128], F32, tag="bv_bc")
            for hl in range(HPC):
                hsl = slice(hl * 128, (hl + 1) * 128)
                nc.sync.dma_start(out=wq_sb[:, hsl], in_=wqT[hl])
                nc.sync.dma_start(out=wk_sb[:, hsl], in_=wkT[hl])
                nc.sync.dma_start(out=wv_sb[:, hsl], in_=wvT[hl])
                nc.sync.dma_start(out=bqk_sb[:, hl:hl + 1], in_=bqs[hl])
                nc.sync.dma_start(out=bqk_sb[:, HPC + hl:HPC + hl + 1], in_=bkc[hl])
                bv_row = cpool.tile([1, 128], F32, tag=f"bv_row{hl}", name=f"bv_row{hl}")
                nc.sync.dma_start(out=bv_row[:], in_=bvr[hl])
                nc.gpsimd.partition_broadcast(bv_bc[:, hsl], bv_row[:])

            dump16 = spool.tile([128, S], BF16, tag="dump16")
            m2dump = spool.tile([128, S // M2_STRIDE], BF16, tag="m2dump")

            ginfo = {}
            pst = {}

            def mk(pi):
                st8 = pst[pi][2]

                def sl(nm):
                    a, bb = _SL[nm]
                    return st8[:, a:bb]

                def slc(nm, i, j=None):
                    a, bb = _SL[nm]
                    if j is None:
                        j = i + 1
                    return st8[:, a + i:a + j]
                return sl, slc

            # ---------------- phase A: proj + scores + m1/m2 raw accums
            def emit_A(pi):
                b, hl = pairs[pi]
                st8 = spool.tile([128, SCOLS], F32, tag=f"stt{pi}",
                                 name=f"stt_p{pi}")
                hs = slice(hl * 128, (hl + 1) * 128)
                xhT = ppool.tile([128, S], BF16, tag="xhT")
                nc.sync.dma_start(out=xhT[:], in_=xT[pi])

                qT = ppool.tile([128, S], BF16, tag="qT")
                kT = ppool.tile([128, S], BF16, tag="kT")
                for ch in range(S // 512):
                    cs = slice(ch * 512, (ch + 1) * 512)
                    ps = psA.tile([128, 512], F32, tag="ps512")
                    nc.tensor.matmul(ps[:], wq_sb[:, hs], xhT[:, cs], start=True, stop=True)
                    nc.scalar.activation(qT[:, cs], ps[:], AF.Identity,
                                         bias=bqk_sb[:, hl:hl + 1], scale=SCALE)
                    ps2 = psA.tile([128, 512], F32, tag="ps512")
                    nc.tensor.matmul(ps2[:], wk_sb[:, hs], xhT[:, cs], start=True, stop=True)
                    nc.scalar.activation(kT[:, cs], ps2[:], AF.Identity,
                                         bias=bqk_sb[:, HPC + hl:HPC + hl + 1], scale=1.0)

                # V projection (fp16, extra ones column for Z)
                v = vpool.tile([128, NT, 129], FP16, tag="v")
                nc.vector.memset(v[:, :, 128:129], 1.0)
                for sb in range(NT):
                    pv = psB.tile([128, 129], F32, tag="pb")
                    nc.tensor.matmul(pv[:, :128], xhT[:, sb * 128:(sb + 1) * 128],
                                     wv_sb[:, hs], start=True, stop=True)
                    nc.vector.tensor_add(v[:, sb, :128], pv[:, :128], bv_bc[:, hs])

                # prefix k-sums for m1-via-PE
                kts = spool.tile([128, NT], F32, tag="kts")
                nc.vector.tensor_reduce(kts[:],
                                        kT[:].rearrange("p (t c) -> p t c", c=128),
                                        axis=AXX, op=ALU.add)
                for t in range(1, NT):
                    nc.gpsimd.tensor_add(kts[:, t:t + 1], kts[:, t:t + 1],
                                         kts[:, t - 1:t])
                kps16 = spool.tile([128, NT], BF16, tag="kps16")
                nc.vector.tensor_copy(kps16[:], kts[:])

                sl, slc = (None, None)
                sc_t = []
                for t in range(NT):
                    sct = scpool.tile([128, 128 * (t + 1)], BF16,
                                      tag=f"sc{t}", name=f"sc{t}_p{pi}")
                    sc_t.append(sct)
                pst[pi] = (sc_t, v, st8)
                ginfo[pi] = (b, hl)
                sl, slc = mk(pi)

                for t in range(NT):
                    W = 128 * (t + 1)
                    stile = sc_t[t]
                    qsl = qT[:, t * 128:(t + 1) * 128]
                    nchk = (W + 511) // 512
                    for ch in range(nchk):
                        c0, c1 = ch * 512, min((ch + 1) * 512, W)
                        ps = psA.tile([128, 512], F32, tag="ps512")
                        last = ch == nchk - 1
                        nc.tensor.matmul(ps[:, :c1 - c0], qsl, kT[:, c0:c1],
                                         start=True, stop=not last)
                        if last:
                            # causal mask: += -1e9 * upper-tri on the diag block
                            d0 = t * 128 - c0
                            nc.tensor.matmul(ps[:, d0:d0 + 128], ident[:], negu[:],
                                             start=False, stop=True)
                        if t in COPY_DVE_TILES:
                            nc.vector.tensor_copy(stile[:, c0:c1], ps[:, :c1 - c0])
                        else:
                            nc.scalar.activation(stile[:, c0:c1], ps[:, :c1 - c0],
                                                 AF.Copy, bias=0.0, scale=1.0)
                    # m1 via PE: q . prefix-ksum, take col t
                    psm = psB.tile([128, 129], F32, tag="pb")
                    nc.tensor.matmul(psm[:, :NT], qsl, kps16[:], start=True, stop=True)
                    nc.vector.tensor_scalar(slc("m1", t), psm[:, t:t + 1],
                                            1.0 / W, None, op0=ALU.mult)
                    # m2 over the pre-diagonal window (valid, unmasked cols)
                    if t >= 1:
                        nsamp = (t * 128) // M2_STRIDE
                        nc.scalar.activation(m2dump[:, :nsamp],
                                             stile[:, 0:t * 128:M2_STRIDE],
                                             AF.Square, bias=0.0, scale=1.0,
                                             accum_out=slc("m2", t))

            # ---------------- stats: sig, shifts, ladder init (Pool + ACT/DVE)
            def emit_stats(pi):
                sl, slc = mk(pi)
                g = nc.gpsimd
                g.tensor_tensor(slc("m2", 1, NT), slc("m2", 1, NT),
                                ct("m2scl")[:, 1:NT], op=ALU.mult)
                g.tensor_tensor(slc("sig", 1, NT), slc("m1", 1, NT),
                                slc("m1", 1, NT), op=ALU.mult)
                g.tensor_tensor(slc("sig", 1, NT), slc("m2", 1, NT),
                                slc("sig", 1, NT), op=ALU.subtract)
                g.tensor_scalar(slc("sig", 1, NT), slc("sig", 1, NT), 1e-6, None,
                                op0=ALU.max)
                nc.scalar.activation(slc("sig", 1, NT), slc("sig", 1, NT),
                                     AF.Sqrt, bias=0.0, scale=1.0)
                g.memset(slc("sig", 0), 1.0)
                nc.vector.reciprocal(sl("invsig"), slc("sig", TSEL, NT))
                # Newton step size 1/density, fixed at the init quantile
                g.tensor_tensor(sl("dinv"), ct("densz"), sl("invsig"), op=ALU.mult)
                g.tensor_scalar(sl("dinv"), sl("dinv"), 15.0, None, op0=ALU.max)
                nc.vector.reciprocal(sl("dinv"), sl("dinv"))

                # keep-all shift tgate = m1 - 4 sig; negc = -max(6, 9 sig - 10.5)
                g.tensor_scalar(sl("tgate"), sl("sig"), -4.0, None, op0=ALU.mult)
                g.tensor_tensor(sl("tgate"), sl("tgate"), sl("m1"), op=ALU.add)
                g.tensor_scalar(sl("negc"), sl("sig"), 9.0, -10.5,
                                op0=ALU.mult, op1=ALU.add)
                g.tensor_scalar(sl("negc"), sl("negc"), 6.0, -1.0,
                                op0=ALU.max, op1=ALU.mult)

                m1s = slc("m1", TSEL, NT)
                sigs = slc("sig", TSEL, NT)
                g.tensor_scalar(sl("Thi"), sigs, 3.0, None, op0=ALU.mult)
                g.tensor_tensor(sl("Thi"), sl("Thi"), m1s, op=ALU.add)
                g.memset(sl("Chi"), 0.0)
                g.tensor_scalar(sl("Tlo"), sigs, -4.0, None, op0=ALU.mult)
                g.tensor_tensor(sl("Tlo"), sl("Tlo"), m1s, op=ALU.add)
                g.memset(sl("Clo"), float(S))
                g.tensor_tensor(sl("Tc"), sigs, ct("ztab"), op=ALU.mult)
                g.tensor_tensor(sl("Tc"), sl("Tc"), m1s, op=ALU.add)

            # ---------------- ladder pieces
            def emit_count(pi):
                sl, slc = mk(pi)
                sc_t = pst[pi][0]
                for i, t in enumerate(range(TSEL, NT)):
                    W = 128 * (t + 1)
                    nc.vector.tensor_scalar(
                        dump16[:, :W], sc_t[t][:], slc("Tc", i), 0.0,
                        op0=ALU.is_ge, op1=ALU.add, accum_out=slc("Cc", i))

            def emit_update(pi):
                # counts are exact small integers in f32, so predicates are
                # arithmetic clamps (GPSIMD codegen rejects compare ALU ops)
                sl, slc = mk(pi)
                g = nc.gpsimd
                Cc, Chi, Clo = sl("Cc"), sl("Chi"), sl("Clo")
                Thi, Tlo, Tc = sl("Thi"), sl("Tlo"), sl("Tc")
                p1, p2, d = sl("p1"), sl("p2"), sl("s2")
                # p1 = (Cc <= K) & (Cc >= Chi)
                g.tensor_scalar(p1, Cc, -1.0, float(K + 1), op0=ALU.mult, op1=ALU.add)
                g.tensor_scalar(p1, p1, 0.0, 1.0, op0=ALU.max, op1=ALU.min)
                g.tensor_tensor(p2, Cc, Chi, op=ALU.subtract)
                g.tensor_scalar(p2, p2, 1.0, 0.0, op0=ALU.add, op1=ALU.max)
                g.tensor_scalar(p2, p2, 1.0, None, op0=ALU.min)
                g.tensor_tensor(p1, p1, p2, op=ALU.mult)
                g.tensor_tensor(d, Tc, Thi, op=ALU.subtract)
                g.tensor_tensor(d, d, p1, op=ALU.mult)
                g.tensor_tensor(Thi, Thi, d, op=ALU.add)
                g.tensor_tensor(d, Cc, Chi, op=ALU.subtract)
                g.tensor_tensor(d, d, p1, op=ALU.mult)
                g.tensor_tensor(Chi, Chi, d, op=ALU.add)
                # p1 = (Cc > K) & (Cc <= Clo)
                g.tensor_scalar(p1, Cc, float(-K), 0.0, op0=ALU.add, op1=ALU.max)
                g.tensor_scalar(p1, p1, 1.0, None, op0=ALU.min)
                g.tensor_tensor(p2, Clo, Cc, op=ALU.subtract)
                g.tensor_scalar(p2, p2, 1.0, 0.0, op0=ALU.add, op1=ALU.max)
                g.tensor_scalar(p2, p2, 1.0, None, op0=ALU.min)
                g.tensor_tensor(p1, p1, p2, op=ALU.mult)
                g.tensor_tensor(d, Tc, Tlo, op=ALU.subtract)
                g.tensor_tensor(d, d, p1, op=ALU.mult)
                g.tensor_tensor(Tlo, Tlo, d, op=ALU.add)
                g.tensor_tensor(d, Cc, Clo, op=ALU.subtract)
                g.tensor_tensor(d, d, p1, op=ALU.mult)
                g.tensor_tensor(Clo, Clo, d, op=ALU.add)

            def emit_newton(pi, aim):
                sl, slc = mk(pi)
                g = nc.gpsimd
                s1, s2, s3 = sl("s1"), sl("s2"), sl("s3")
                p1, p2 = sl("p1"), sl("p2")
                Tc, Cc = sl("Tc"), sl("Cc")
                g.tensor_scalar(s2, Cc, float(aim), None, op0=ALU.subtract)
                g.tensor_tensor(s2, s2, sl("dinv"), op=ALU.mult)
                g.tensor_tensor(s3, Tc, s2, op=ALU.add)
                # clamp into the bracket with a 10% margin: an overshooting
                # Newton step becomes a near-edge probe (damped bisection).
                # Pool TT max/min are illegal -> relu-style arithmetic clamps.
                g.tensor_tensor(p1, sl("Thi"), sl("Tlo"), op=ALU.subtract)
                g.tensor_scalar(p1, p1, 0.1, None, op0=ALU.mult)
                g.tensor_tensor(p2, sl("Tlo"), p1, op=ALU.add)
                g.tensor_tensor(s3, s3, p2, op=ALU.subtract)
                g.tensor_scalar(s3, s3, 0.0, None, op0=ALU.max)
                g.tensor_tensor(s3, s3, p2, op=ALU.add)      # = max(s3, Tlo+m)
                g.tensor_tensor(p2, sl("Thi"), p1, op=ALU.subtract)
                g.tensor_tensor(s3, p2, s3, op=ALU.subtract)
                g.tensor_scalar(s3, s3, 0.0, None, op0=ALU.max)
                g.tensor_tensor(Tc, p2, s3, op=ALU.subtract)  # = min(s3, Thi-m)

            def emit_bisect(pi):
                sl, slc = mk(pi)
                g = nc.gpsimd
                g.tensor_tensor(sl("s3"), sl("Thi"), sl("Tlo"), op=ALU.add)
                g.tensor_scalar(sl("Tc"), sl("s3"), 0.5, None, op0=ALU.mult)

            def emit_finalize(pi):
                sl, slc = mk(pi)
                g = nc.gpsimd
                # threshold = Thi if Chi >= 1 else Tlo (never keep 0 entries)
                g.tensor_scalar(sl("p1"), sl("Chi"), 0.0, 1.0,
                                op0=ALU.max, op1=ALU.min)
                g.tensor_tensor(sl("s2"), sl("Thi"), sl("Tlo"), op=ALU.subtract)
                g.tensor_tensor(sl("s2"), sl("s2"), sl("p1"), op=ALU.mult)
                g.tensor_tensor(slc("tgate", TSEL, NT), sl("Tlo"), sl("s2"),
                                op=ALU.add)
                # exp bias = negc - tgate
                g.tensor_tensor(sl("bias"), sl("negc"), sl("tgate"),
                                op=ALU.subtract)

            def emit_ladder(gpis):
                nrungs = 1 + len(NEWTON_AIMS) + NBISECT
                for rung in range(nrungs):
                    for pi in gpis:
                        emit_count(pi)
                        emit_update(pi)
                        if rung < len(NEWTON_AIMS):
                            emit_newton(pi, NEWTON_AIMS[rung])
                        elif rung < nrungs - 1:
                            emit_bisect(pi)
                        else:
                            emit_finalize(pi)

            # ---------------- phase C split into front/back stages
            # front(t): exp + mask + mult + transpose (ACT/DVE/Pool/SP)
            # back(t):  AV matmuls + Z-recip + rescale  (PE/DVE)
            cstate = {}

            def emit_C_front(pi, t):
                sl, slc = mk(pi)
                sc_t = pst[pi][0]
                W = 128 * (t + 1)
                stile = sc_t[t]
                et = rpool.tile([128, S], FP16, tag="et")
                nc.scalar.activation(et[:, :W], stile[:], AF.Exp,
                                     bias=slc("bias", t), scale=1.0)
                if t >= TSEL:
                    msk = mskpool.tile([128, S], FP16, tag="msk")
                    nc.vector.tensor_scalar(msk[:, :W], stile[:],
                                            slc("tgate", t), None, op0=ALU.is_ge)
                    eng = nc.vector if t in MULT_DVE_TILES else nc.gpsimd
                    eng.tensor_mul(et[:, :W], et[:, :W], msk[:, :W])
                aT = rpool.tile([128, NT, 128], FP16, tag="aT")
                nc.sync.dma_start_transpose(aT[:, :t + 1, :], et[:, :W])
                cstate[(pi, t)] = aT

            ohmap = {}

            def emit_C_back(pi, t):
                sl, slc = mk(pi)
                v = pst[pi][1]
                if pi not in ohmap:
                    ohmap[pi] = ohpool.tile([128, NT, 128], BF16, tag="outh",
                                            name=f"outh_p{pi}")
                out_h = ohmap[pi]
                aT = cstate.pop((pi, t))
                po = psB.tile([128, 129], F32, tag="pb")
                for kb in range(t + 1):
                    nc.tensor.matmul(po[:], aT[:, kb, :], v[:, kb, :],
                                     start=(kb == 0), stop=(kb == t))
                nc.vector.tensor_copy(slc("zc", 0), po[:, 128:129])
                nc.vector.reciprocal(slc("rz", t), slc("zc", 0))
                nc.vector.tensor_scalar(out_h[:, t, :], po[:, 0:128],
                                        slc("rz", t), None, op0=ALU.mult)

            def emit_LC(lpi, cpi, oproj=None):
                """Ladder of lpi stitched with phase C of cpi and optional
                out_proj chunks (oproj = list of pair indices)."""
                nrungs = 1 + len(NEWTON_AIMS) + NBISECT
                steps = []
                if cpi is not None:
                    for t in range(NT + 1):
                        steps.append(("c", t))
                if oproj is not None:
                    ostep = max(1, len(steps) // NCH) if steps else 1
                    merged = []
                    och = 0
                    for i, st in enumerate(steps):
                        merged.append(st)
                        if (i + 1) % ostep == 0 and och < NCH:
                            merged.append(("o", och))
                            och += 1
                    while och < NCH:
                        merged.append(("o", och))
                        och += 1
                    steps = merged

                def do_step(st):
                    kind, i = st
                    if kind == "c":
                        if i < NT:
                            emit_C_front(cpi, i)
                        if i >= 1:
                            emit_C_back(cpi, i - 1)
                    else:
                        emit_oproj_chunk(oproj, i)

                if lpi is None:
                    for st in steps:
                        do_step(st)
                    return
                per = (len(steps) + nrungs - 1) // nrungs if steps else 0
                idx = 0
                for rung in range(nrungs):
                    emit_count(lpi)
                    emit_update(lpi)
                    if rung < len(NEWTON_AIMS):
                        emit_newton(lpi, NEWTON_AIMS[rung])
                    elif rung < nrungs - 1:
                        emit_bisect(lpi)
                    else:
                        emit_finalize(lpi)
                    for st in steps[idx:idx + per]:
                        do_step(st)
                    idx += per
                for st in steps[idx:]:
                    do_step(st)

            # ---------------- out_proj, one chunk at a time (stitchable)
            def emit_oproj_chunk(group_pis, ch):
                cs = slice(ch * CW, (ch + 1) * CW)
                wo_t = wopool.tile([128, NT, CW], BF16, tag="wo_t")
                nc.gpsimd.dma_start(out=wo_t[:], in_=woT[:, ch])
                for pi in group_pis:
                    b, hl = ginfo[pi]
                    out_h = ohmap[pi]
                    pg = psA.tile([128, 512], F32, tag="ps512")
                    for sb in range(NT):
                        nc.tensor.matmul(pg[:, :CW], out_h[:, sb, :],
                                         wo_t[:, sb, :],
                                         start=(sb == 0), stop=(sb == NT - 1))
                    yt = rpool.tile([128, CW], F32, tag="yt")
                    nc.vector.tensor_add(yt[:], pg[:, :CW], bo_bc[:, cs])
                    nc.sync.dma_start(out=y[b, hl * 128:(hl + 1) * 128, cs],
                                      in_=yt[:])

            # ---------------- main schedule: depth-1 software pipeline with
            # ladder(i+1) stitched into phase-C(i) tile steps
            for _rep in range(BODY_REPS):
                ginfo.clear()
                pst.clear()
                cstate.clear()
                ohmap.clear()
                emit_A(0)
                emit_stats(0)
                emit_A(1)
                emit_LC(0, None)
                emit_stats(1)
                emit_LC(1, 0)
                emit_A(2)
                emit_stats(2)
                emit_LC(2, 1)
                emit_A(3)
                emit_stats(3)
                emit_LC(3, 2, oproj=[0, 1])
                emit_LC(None, 3)
                for ch in range(NCH):
                    emit_oproj_chunk([2, 3], ch)

    nc.compile()
    return nc, {}


# ---------------------------------------------------------------- host side

_NC_CACHE = {}


def get_nc():
    if "nc" not in _NC_CACHE:
        _NC_CACHE["nc"] = build_nc()
    return _NC_CACHE["nc"]


def host_prep(x, Wq, Wk, Wv, bq, bk, bv, Wo, bo):
    ctab, _ = _get_ctab()
    bf = ml_dtypes.bfloat16
    # woTr[p, ch, bb, c] = Wo.T[bb*128+p, ch*CW+c]
    woT = np.ascontiguousarray(
        Wo.T.reshape(NT, 128, NCH, CW).transpose(1, 2, 0, 3).astype(bf))
    ident = np.eye(128, dtype=np.float32).astype(bf)
    negu = np.triu(np.full((128, 128), NEGBIG, np.float32), 1).astype(bf)
    in_maps = []
    pairs = [(b, hl) for hl in range(HPC) for b in range(B)]
    for c in range(NCORES):
        heads = [HPC * c + i for i in range(HPC)]
        xTs = np.empty((NPAIR, 128, S), bf)
        for pi, (b, hl) in enumerate(pairs):
            h = heads[hl]
            xTs[pi] = np.ascontiguousarray(
                x[b, :, h * HD:(h + 1) * HD].T).astype(bf)
        m = dict(
            xT=xTs,
            wqT=np.ascontiguousarray(
                np.stack([Wq[h].T for h in heads])).astype(bf),
            wkT=np.ascontiguousarray(
                np.stack([Wk[h].T for h in heads])).astype(bf),
            wvT=np.ascontiguousarray(
                np.stack([Wv[h].T for h in heads])).astype(bf),
            bqs=np.ascontiguousarray(
                (np.stack([bq[h] for h in heads]) * SCALE)[:, :, None].astype(np.float32)),
            bkc=np.ascontiguousarray(
                np.stack([bk[h] for h in heads])[:, :, None].astype(np.float32)),
            bvr=np.ascontiguousarray(
                np.stack([bv[h] for h in heads])[:, None, :].astype(np.float32)),
            woT=woT,
            bor=np.ascontiguousarray(bo[None, :]).astype(bf),
            ident=ident,
            negu=negu,
            ctab=ctab,
        )
        in_maps.append(m)
    return in_maps


def kernel(x, causal_mask, Wq, Wk, Wv, bq, bk, bv, Wo, bo):
    nc, _dbg = get_nc()
    in_maps = host_prep(np.asarray(x), np.asarray(Wq), np.asarray(Wk),
                        np.asarray(Wv), np.asarray(bq), np.asarray(bk),
                        np.asarray(bv), np.asarray(Wo), np.asarray(bo))
    res = run_bass_kernel_spmd(nc, in_maps, list(range(NCORES)))
    y = np.empty((B, DIM, S), np.float32)
    for c in range(NCORES):
        y[:, c * HPC * HD:(c + 1) * HPC * HD, :] = res.results[c]["y"]
    return y
